# revision 30
# baseline (speedup 1.0000x reference)
# Trainium2 Bass kernel for LocLoss: per-sample argmax over a 192x192 cls map,
# gather of loc values at the argmax position, smooth-L1 loss vs a
# center_rate-derived bias, mean-reduced.
#
# Sharding: pure data parallel, batch 256 -> 8 cores x 32 samples.
# Per-core layout: the 36864-element cls map of sample s is split into 4
# chunks of 48 rows; partition p = s*4 + ch holds chunk ch. One bulk
# reduce_max pass produces per-row maxes; everything after operates on tiny
# (32, k) tiles. loc is never read in bulk: the 2 needed values per sample
# are fetched with an indirect DMA gather at the computed (r, c).
import numpy as np
from contextlib import ExitStack

import concourse.bass as bass
import concourse.bacc as bacc
import concourse.mybir as mybir
import concourse.tile as tile

B = 256
NCORES = 8
BP = B // NCORES          # 32 samples per core
H = W = 192
MAP = H * W               # 36864
NCHUNK = 4                # chunks per sample -> 128 partitions
ROWS_PER_PART = H // NCHUNK   # 48
CHUNK = ROWS_PER_PART * W     # 9216
NSLICE = 6                # streaming slices of the bulk cls load
SL_ROWS = ROWS_PER_PART // NSLICE   # 8 rows per partition per slice
SL_ELEMS = SL_ROWS * W              # 1536

F32 = mybir.dt.float32
U32 = mybir.dt.uint32
I32 = mybir.dt.int32
ALU = mybir.AluOpType


def build_program(with_dbg=False):
    nc = bacc.Bacc("TRN2", target_bir_lowering=False, debug=False, num_devices=NCORES)

    # cls as (rows, W): row index = s*192 + r, contiguous with host (32, 36864)
    cls_d = nc.dram_tensor("cls", [BP * H, W], F32, kind="ExternalInput")
    # host-shuffled copy in (ch, s, chunk) order: the bulk load for partition
    # p = ch*BP + s streams sequential DRAM with 9-36KB descriptors
    cls_shuf_d = nc.dram_tensor("cls_shuf", [128, CHUNK], F32, kind="ExternalInput")
    # loc host-transposed to (s, pos, ch) so both channel values at a map
    # position are adjacent: one indirect-gather index per sample fetches 2
    # contiguous elements (HW DGE gathers use one index per partition).
    loc_d = nc.dram_tensor("loc", [BP * MAP * 2 // 2048, 2048], F32,
                           kind="ExternalInput")
    cr_d = nc.dram_tensor("cr", [BP, 2], F32, kind="ExternalInput")
    loss_d = nc.dram_tensor("loss", [BP, 2], F32, kind="ExternalOutput")
    dbg_d = (nc.dram_tensor("dbg", [BP, 8], F32, kind="ExternalOutput")
             if with_dbg else None)

    with tile.TileContext(nc) as tc:
        with ExitStack() as ctx:
            const = ctx.enter_context(tc.tile_pool(name="const", bufs=1))
            stream = ctx.enter_context(tc.tile_pool(name="stream", bufs=3))
            small = ctx.enter_context(tc.tile_pool(name="small", bufs=1))

            cls_view = cls_shuf_d[:]  # (128, 9216), p = ch*BP + s

            # --- bulk pass: per-(partition, row) max -> (128, 48)
            # SWDGE (gpsimd) DMAs round-robin across 8 queues -> 16 SDMA
            # engines; HWDGE queues all pin to the same 4 engines. Slices
            # shrink toward the end so the final reduce trails the last
            # (tiny, low-latency HWDGE) load by well under 1us.
            # HWDGE (sync/scalar) queues start issuing ~3.5us before the
            # SWDGE ring, so they take the leading slices; gpsimd carries
            # the bulk at full multi-queue rate
            slices = [(nc.sync, 9), (nc.scalar, 7), (nc.gpsimd, 10),
                      (nc.gpsimd, 10), (nc.gpsimd, 10), (nc.sync, 1),
                      (nc.gpsimd, 1)]
            row_max = const.tile([128, ROWS_PER_PART], F32)
            r0 = 0
            for i, (eng, nrows) in enumerate(slices):
                t = stream.tile([128, nrows * W], F32, tag=f"cls_slice{i}")
                eng.dma_start(t[:], cls_view[:, r0 * W:(r0 + nrows) * W])
                nc.vector.reduce_max(
                    row_max[:, r0:r0 + nrows],
                    t[:].rearrange("p (a c) -> p a c", c=W),
                    axis=mybir.AxisListType.X,
                )
                r0 += nrows

            # --- per-sample row maxes: rowT[s, r] over all 192 global rows
            rowT = small.tile([BP, H], F32)
            rowt_engines = [nc.sync, nc.scalar, nc.gpsimd, nc.sync]
            for ch in range(NCHUNK):
                rowt_engines[ch].dma_start(
                    rowT[:, ch * ROWS_PER_PART:(ch + 1) * ROWS_PER_PART],
                    row_max[ch * BP:(ch + 1) * BP, :],
                )

            m8 = small.tile([BP, 8], F32)
            ri8 = small.tile([BP, 8], U32)
            nc.vector.max(out=m8[:], in_=rowT[:])
            nc.vector.max_index(out=ri8[:], in_max=m8[:], in_values=rowT[:])

            r_f = small.tile([BP, 1], F32)
            nc.vector.tensor_copy(r_f[:], ri8[:, 0:1])

            # global row index into cls_d: s*192 + r
            s192_i = small.tile([BP, 1], I32)
            nc.gpsimd.iota(s192_i[:], pattern=[[1, 1]], base=0, channel_multiplier=H)
            s192_f = small.tile([BP, 1], F32)
            nc.vector.tensor_copy(s192_f[:], s192_i[:])
            rowidx_f = small.tile([BP, 1], F32)
            nc.vector.tensor_tensor(rowidx_f[:], r_f[:], s192_f[:], op=ALU.add)
            rowidx_u = small.tile([BP, 1], U32)
            nc.vector.tensor_copy(rowidx_u[:], rowidx_f[:])

            # gather each sample's winning row (192 f32) from DRAM
            rows_t = small.tile([BP, W], F32)
            nc.gpsimd.indirect_dma_start(
                out=rows_t[:],
                out_offset=None,
                in_=cls_d[:],
                in_offset=bass.IndirectOffsetOnAxis(ap=rowidx_u[:, 0:1], axis=0),
            )

            rm8 = small.tile([BP, 8], F32)
            ci8 = small.tile([BP, 8], U32)
            nc.vector.max(out=rm8[:], in_=rows_t[:])
            nc.vector.max_index(out=ci8[:], in_max=rm8[:], in_values=rows_t[:])
            c_f = small.tile([BP, 1], F32)
            nc.vector.tensor_copy(c_f[:], ci8[:, 0:1])

            # loc flat element offsets: off[s, ch] = s*73728 + ch*36864 + r*192 + c
            # element offset = 2*(s*36864 + r*192 + c); iota gives 2*s,
            # scaled by 36864 (iota pattern steps are int16-bound)
            base_i = small.tile([BP, 1], I32)
            nc.gpsimd.iota(base_i[:], pattern=[[1, 1]], base=0,
                           channel_multiplier=2)
            base_f = small.tile([BP, 1], F32)
            nc.vector.tensor_copy(base_f[:], base_i[:])
            nc.vector.tensor_scalar_mul(base_f[:], base_f[:], float(MAP))

            rc_f = small.tile([BP, 1], F32)
            nc.vector.tensor_scalar(rc_f[:], r_f[:], float(W), c_f[:, 0:1],
                                    op0=ALU.mult, op1=ALU.add)
            off_f = small.tile([BP, 1], F32)
            nc.vector.scalar_tensor_tensor(off_f[:], rc_f[:], 2.0, base_f[:],
                                           op0=ALU.mult, op1=ALU.add)
            off_u = small.tile([BP, 1], U32)
            nc.vector.tensor_copy(off_u[:], off_f[:])

            loc_pos = small.tile([BP, 2], F32)
            nc.gpsimd.indirect_dma_start(
                out=loc_pos[:],
                out_offset=None,
                in_=loc_d[:],
                in_offset=bass.IndirectOffsetOnAxis(ap=off_u[:, 0:1], axis=1),
            )

            # bias = center_rate*191 - [r, c]
            cr_t = small.tile([BP, 2], F32)
            nc.sync.dma_start(cr_t[:], cr_d[:])
            rc2 = small.tile([BP, 2], F32)
            nc.vector.tensor_copy(rc2[:, 0:1], r_f[:])
            nc.vector.tensor_copy(rc2[:, 1:2], c_f[:])
            bias = small.tile([BP, 2], F32)
            nc.vector.tensor_scalar(bias[:], cr_t[:], float(H - 1), None,
                                    op0=ALU.mult)
            nc.vector.tensor_tensor(bias[:], bias[:], rc2[:], op=ALU.subtract)

            # smooth L1 (beta=1)
            diff = small.tile([BP, 2], F32)
            nc.vector.tensor_tensor(diff[:], loc_pos[:], bias[:], op=ALU.subtract)
            ad = small.tile([BP, 2], F32)
            nc.scalar.activation(ad[:], diff[:], mybir.ActivationFunctionType.Abs)
            quad = small.tile([BP, 2], F32)
            nc.vector.scalar_tensor_tensor(quad[:], ad[:], 0.5, ad[:],
                                           op0=ALU.mult, op1=ALU.mult)
            lin = small.tile([BP, 2], F32)
            nc.vector.tensor_scalar_add(lin[:], ad[:], -0.5)
            mlt = small.tile([BP, 2], F32)
            nc.vector.tensor_scalar(mlt[:], ad[:], 1.0, None, op0=ALU.is_lt)
            # lval = lin + mlt*(quad - lin)
            tsel = small.tile([BP, 2], F32)
            nc.vector.tensor_tensor(tsel[:], quad[:], lin[:], op=ALU.subtract)
            nc.vector.tensor_tensor(tsel[:], mlt[:], tsel[:], op=ALU.mult)
            lval = small.tile([BP, 2], F32)
            nc.vector.tensor_tensor(lval[:], lin[:], tsel[:], op=ALU.add)

            nc.sync.dma_start(loss_d[:], lval[:])

            if with_dbg:
                dbg = small.tile([BP, 8], F32)
                nc.vector.tensor_copy(dbg[:, 0:1], m8[:, 0:1])
                nc.vector.tensor_copy(dbg[:, 1:2], r_f[:])
                nc.vector.tensor_copy(dbg[:, 2:3], c_f[:])
                nc.vector.tensor_copy(dbg[:, 3:5], loc_pos[:])
                nc.vector.tensor_copy(dbg[:, 5:7], bias[:])
                nc.vector.tensor_copy(dbg[:, 7:8], rm8[:, 0:1])
                nc.sync.dma_start(dbg_d[:], dbg[:])

    nc.compile()
    return nc


_NC_CACHE = None


def _get_program():
    global _NC_CACHE
    if _NC_CACHE is None:
        _NC_CACHE = build_program()
    return _NC_CACHE


def make_in_maps(cls_input, loc_input, center_rate):
    cls = np.ascontiguousarray(np.asarray(cls_input, dtype=np.float32)).reshape(
        NCORES, BP * H, W)
    cls_shuf = np.ascontiguousarray(
        cls.reshape(NCORES, BP, NCHUNK, CHUNK).transpose(0, 2, 1, 3)).reshape(
        NCORES, 128, CHUNK)
    loc = np.asarray(loc_input, dtype=np.float32).reshape(B, 2, MAP)
    loc = np.ascontiguousarray(loc.transpose(0, 2, 1)).reshape(
        NCORES, BP * MAP * 2 // 2048, 2048)
    cr = np.ascontiguousarray(np.asarray(center_rate, dtype=np.float32)).reshape(
        NCORES, BP, 2)
    return [
        {"cls": cls[c], "cls_shuf": cls_shuf[c], "loc": loc[c], "cr": cr[c]}
        for c in range(NCORES)
    ]


def kernel(cls_input, loc_input, center_rate, _trace=False, _results_out=None):
    from concourse.bass_utils import run_bass_kernel_spmd

    nc = _get_program()
    in_maps = make_in_maps(cls_input, loc_input, center_rate)
    res = run_bass_kernel_spmd(nc, in_maps, list(range(NCORES)), trace=_trace)
    if _results_out is not None:
        _results_out.append(res)
    losses = np.concatenate([r["loss"] for r in res.results], axis=0)  # (256, 2)
    return np.float32(np.mean(losses, dtype=np.float64))
